# revision 29
# baseline (speedup 1.0000x reference)
"""Trainium2 Bass kernel for nn_CT_loss (data-parallel over batch, 8 cores).

v4: tensor-engine-centric. Per batch:
  u = A p + b0, c = G p + g0 (A = R diag(e), G = R^T A), vt_ai = v_ai/s_a,
  d_ai = u_i - c_a vt_ai, w_a = sum_i d^2, out(b,a) = sum_pix sqrt(w) mask.
Host multiplies per-(b,a) sums by |s_a|, applies gating + normalization.

Key identity: c_a*vt_ai = alpha[a,i]*(c_a Q_A) + beta[a,i]*(c_a Q_B)
              + h[a,i]*c_a, and h*c folds into the P0 weights (c = G p + g0).
So the only per-pixel elementwise product is y = crep2 (.) [Q_A; Q_B]
(PSUM x SBUF, one DVE op); everything else is matmuls.

Per chunk of 512 pixels (32 chunks/core), channel-major rows cls*8+b:
  MM_c: [25->48]  crep2 rows (side, a, b)       from P0+ones (XP tile)
  DVE:  y = crep2 (.) XQ[0:48] -> RHSD[0:48]    (Q_A | Q_B channel blocks)
  MM_d: [89->72]  d = u' - alpha y_A - beta y_B  (y + pad + P0 + ones)
  ACT:  SQ = Square(d) -> bf16
  MM_r: [72->128] w_a rows 32*slot + a*8 + b; 4 chunks accumulate into one
        [128,512] PSUM bank via slot-shifted weight matrices (start/stop)
  per group: ACT Sqrt -> la; DVE tensor_tensor_reduce(la*mask) -> ACC[:, g]

All matmuls are plain tile_position=(0,0); engine APs all base-0
(non-zero partition bases / col-group tiling are broken on this setup).
"""
import os
import sys

import numpy as np

for _p in ("/opt/trn_rl_repo",):
    if _p not in sys.path:
        sys.path.insert(0, _p)

import concourse.bass as bass
import concourse.bacc as bacc
import concourse.tile as tile
from concourse import mybir
from concourse.bass_utils import run_bass_kernel_spmd

from ml_dtypes import bfloat16

F32 = mybir.dt.float32
BF16 = mybir.dt.bfloat16
AF = mybir.ActivationFunctionType
OP = mybir.AluOpType

B, HW = 64, 128 * 128
NCORES, BPC = 8, 8
NCH = 512                  # pixels per chunk (one PSUM bank of fp32)
NCHUNKS = HW // NCH        # 32 chunks per core
NG = NCHUNKS // 4          # 8 groups of 4 chunks

# a -> (c1, c2, qA, qB): Q channel indices per direction
QCH = {0: (1, 2, 0, 1), 1: (0, 2, 2, 3), 2: (0, 1, 4, 5)}
QA_ORDER = [QCH[a][2] for a in range(3)]   # [0, 2, 4]
QB_ORDER = [QCH[a][3] for a in range(3)]   # [1, 3, 5]

_BUILT = None
LAST = None


def _build_nc():
    nc = bacc.Bacc(None)
    xq = nc.dram_tensor("xq", [48, HW], BF16, kind="ExternalInput")
    xz = nc.dram_tensor("xz", [25, HW], BF16, kind="ExternalInput")
    mk = nc.dram_tensor("mk", [128, NG * NCH], BF16, kind="ExternalInput")
    wc_d = nc.dram_tensor("wc", [25, 48], BF16, kind="ExternalInput")
    wd_d = nc.dram_tensor("wd", [73, 72], BF16, kind="ExternalInput")
    wr_d = nc.dram_tensor("wr", [72, 512], BF16, kind="ExternalInput")
    outp = nc.dram_tensor("out", [128, NG], F32, kind="ExternalOutput")

    with tile.TileContext(nc) as tc:
        with tc.tile_pool(name="big", bufs=1) as big, \
             tc.tile_pool(name="sq", bufs=4) as sqp, \
             tc.tile_pool(name="la", bufs=2) as lap, \
             tc.tile_pool(name="lm", bufs=2) as lmp, \
             tc.tile_pool(name="pc", bufs=2, space="PSUM") as pcp, \
             tc.tile_pool(name="pd", bufs=3, space="PSUM") as pdp, \
             tc.tile_pool(name="pwm", bufs=1, space="PSUM") as pwmp, \
             tc.tile_pool(name="pw", bufs=2, space="PSUM") as pwp:
            WT = big.tile([73, 72 + 48], BF16, tag="WT")
            nc.scalar.dma_start(WT[0:73, 0:72], wd_d[:])
            nc.scalar.dma_start(WT[0:25, 72:120], wc_d[:])
            WRT = big.tile([72, 512], BF16, tag="WRT")
            nc.scalar.dma_start(WRT[:], wr_d[:])

            # PE warm-up during the DMA head: ~8us of dummy matmuls flips
            # the HAM clock gate to 8/8 (2.4 GHz) before real work arrives
            WARM = pwmp.tile([128, NCH], F32, tag="WARM")
            for i in range(24):
                nc.tensor.matmul(WARM[:], WRT[:, 128 * (i % 4):128 * (i % 4) + 128],
                                 WRT[:], start=True, stop=True)

            XQ = big.tile([48, HW], BF16, tag="XQ")
            XP = big.tile([25, HW], BF16, tag="XP")
            RHSD = big.tile([73, HW], BF16, tag="RHSD")
            MASK = big.tile([128, NG * NCH], BF16, tag="MASK")
            # column-piece loads so chunk-0 compute starts early and DMA
            # overlaps compute; alternate the two HWDGE rings
            NP = 8
            PW = HW // NP
            for p in range(NP):
                ps = slice(p * PW, (p + 1) * PW)
                e0 = nc.sync if p % 2 == 0 else nc.scalar
                e1 = nc.scalar if p % 2 == 0 else nc.sync
                e0.dma_start(XQ[:, ps], xq[:, ps])
                e1.dma_start(RHSD[48:73, ps], xz[:, ps])
                e0.dma_start(XP[:, ps], xz[:, ps])
                ms = slice(p * NCH, (p + 1) * NCH)
                e1.dma_start(MASK[:, ms], mk[:, ms])
            ACC = big.tile([128, NG], F32, tag="ACC")
            TTD = big.tile([128, NCH], BF16, tag="TTD")

            Wd = WT[0:73, 0:72]
            Wc = WT[0:25, 72:120]

            # software-pipelined emission (2-chunk skew) so each PE
            # instruction's inputs are ready well ahead -> PE runs dense,
            # HAM un-throttles to 2.4 GHz
            w = None
            sqt = [None] * NCHUNKS
            dt_ = [None] * NCHUNKS
            for t in range(NCHUNKS + 2):
                if t < NCHUNKS:
                    c = t
                    cs = slice(c * NCH, (c + 1) * NCH)
                    C = pcp.tile([48, NCH], F32, tag="C")
                    nc.tensor.matmul(C[:], Wc, XP[:, cs], start=True,
                                     stop=True)
                    nc.vector.tensor_mul(RHSD[0:48, cs], C[:], XQ[:, cs])
                if 1 <= t <= NCHUNKS:
                    c = t - 1
                    cs = slice(c * NCH, (c + 1) * NCH)
                    D = pdp.tile([72, NCH], F32, tag="D")
                    dt_[c] = D
                    nc.tensor.matmul(D[:], Wd, RHSD[:, cs], start=True,
                                     stop=True)
                    sq = sqp.tile([72, NCH], BF16, tag="sq")
                    sqt[c] = sq
                    nc.scalar.activation(sq[:], D[:], AF.Square)
                if t >= 2:
                    c = t - 2
                    g, slot = divmod(c, 4)
                    gs = slice(g * NCH, (g + 1) * NCH)
                    if slot == 0:
                        w = pwp.tile([128, NCH], F32, tag="W")
                    nc.tensor.matmul(w[:], WRT[:, 128 * slot:128 * slot + 128],
                                     sqt[c][:], start=(slot == 0),
                                     stop=(slot == 3))
                    sqt[c] = None
                    if slot == 3:
                        la = lap.tile([128, NCH], BF16, tag="la")
                        nc.scalar.activation(la[:], w[:], AF.Sqrt)
                        lm = lmp.tile([128, NCH], BF16, tag="lm")
                        nc.vector.tensor_mul(lm[:], la[:], MASK[:, gs])
                        nc.scalar.activation(TTD[:], lm[:], AF.Identity,
                                             accum_out=ACC[:, g:g + 1])

            nc.sync.dma_start(outp[:], ACC[:])

    nc.compile()
    return nc


def get_nc():
    global _BUILT
    if _BUILT is None:
        _BUILT = _build_nc()
    return _BUILT


def host_constants(R, T, E):
    """Per-core weight matrices (fp64 host math -> bf16)."""
    wc = np.zeros((NCORES, 25, 48), np.float64)
    wd = np.zeros((NCORES, 73, 72), np.float64)
    wr = np.zeros((NCORES, 72, 512), np.float64)
    sabs = np.zeros((B, 3), np.float64)
    for gb in range(B):
        k, b = divmod(gb, BPC)
        Rb = R[gb].astype(np.float64)
        tb = T[gb].astype(np.float64)
        eb = E[gb].astype(np.float64)
        A = Rb * eb[None, :]
        b0 = tb - 0.5 * (Rb @ eb)
        G = Rb.T @ A
        g0 = Rb.T @ b0
        s = Rb.T @ tb
        for a in range(3):
            c1, c2, _, _ = QCH[a]
            sh = np.sign(s[a]) * max(abs(s[a]), 1e-12) if s[a] != 0 else 1e-12
            sabs[gb, a] = abs(sh)
            # crep2 columns: side*24 + a*8 + b
            for side in range(2):
                r2 = side * 24 + a * 8 + b
                for j in range(3):
                    wc[k, j * 8 + b, r2] = G[a, j]
                wc[k, 24, r2] = g0[a]
            for i in range(3):
                r = (3 * a + i) * 8 + b
                al = A[i, c1] / sh
                be = A[i, c2] / sh
                h = (tb[i] - 0.5 * (A[i, c1] + A[i, c2])) / sh
                wd[k, a * 8 + b, r] = -al
                wd[k, 24 + a * 8 + b, r] = -be
                for j in range(3):
                    wd[k, 48 + j * 8 + b, r] = A[i, j] - h * G[a, j]
                wd[k, 72, r] = b0[i] - h * g0[a]
                for slot in range(4):
                    wr[k, r, 128 * slot + 32 * slot + a * 8 + b] = 1.0
    return (wc.astype(bfloat16), wd.astype(bfloat16),
            wr.astype(bfloat16), sabs)


def make_in_maps(P0, Q0, M, wc, wd, wr):
    in_maps = []
    for k in range(NCORES):
        sl = slice(k * BPC, (k + 1) * BPC)
        # xq rows: 0..23 Q_A channels [0,2,4] (block a), 24..47 Q_B [1,3,5]
        q = Q0[sl].reshape(BPC, 6, HW)
        xq = np.empty((48, HW), np.float32)
        xq[0:24] = q[:, QA_ORDER].transpose(1, 0, 2).reshape(24, HW)
        xq[24:48] = q[:, QB_ORDER].transpose(1, 0, 2).reshape(24, HW)
        # xz: rows 0..23 P0 rows j*8+b, 24 ones
        xz = np.zeros((25, HW), np.float32)
        xz[0:24] = P0[sl].reshape(BPC, 3, HW).transpose(1, 0, 2).reshape(24, HW)
        xz[24] = 1.0
        # mask rows 32*slot + a*8 + b, cols g*NCH+p <- M[b,a,(4g+slot)*NCH+p]
        mkc = np.zeros((4, 32, NG, NCH), np.float32)  # slot, row, g, p
        msl = M[sl].reshape(BPC, 3, NG, 4, NCH)       # b a g slot p
        mkc[:, 0:24] = msl.transpose(3, 1, 0, 2, 4).reshape(4, 24, NG, NCH)
        mk = mkc.reshape(128, NG * NCH)
        in_maps.append({
            "xq": xq.astype(bfloat16), "xz": xz.astype(bfloat16),
            "mk": mk.astype(bfloat16),
            "wc": np.ascontiguousarray(wc[k]),
            "wd": np.ascontiguousarray(wd[k]),
            "wr": np.ascontiguousarray(wr[k]),
        })
    return in_maps


def kernel(pred_rots, pred_P0, pred_Q0, gt_occmask, roi_extent, pred_transes):
    global LAST
    R = np.asarray(pred_rots, np.float32)
    P0 = np.asarray(pred_P0, np.float32)
    Q0 = np.asarray(pred_Q0, np.float32)
    M = np.asarray(gt_occmask, np.float32)
    E = np.asarray(roi_extent, np.float32)
    T = np.asarray(pred_transes, np.float32)

    nc = get_nc()
    wc, wd, wr, sabs = host_constants(R, T, E)
    in_maps = make_in_maps(P0, Q0, M, wc, wd, wr)
    trace = os.environ.get("KERNEL_TRACE", "0") == "1"
    LAST = run_bass_kernel_spmd(nc, in_maps, core_ids=list(range(NCORES)),
                                trace=trace)
    S = np.zeros((B, 3), np.float64)
    for k, r in enumerate(LAST.results):
        acc = r["out"].astype(np.float64).sum(axis=1).reshape(4, 32)
        rows = acc[:, 0:24].sum(axis=0)              # over slots
        S[k * BPC:(k + 1) * BPC] += rows.reshape(3, 8).T   # b, a
    S *= sabs
    Msum_a = M.sum(axis=(0, 2, 3)).astype(np.float64)  # per-a mask sums
    loss = sum(S[:, a].sum() for a in range(3) if Msum_a[a] >= 3 * B)
    total = max(Msum_a.sum(), 1.0)
    return np.asarray(np.float32(loss / total))


# revision 33
# speedup vs baseline: 1.0765x; 1.0765x over previous
"""Trainium2 Bass kernel for nn_CT_loss (data-parallel over batch, 8 cores).

v4: tensor-engine-centric. Per batch:
  u = A p + b0, c = G p + g0 (A = R diag(e), G = R^T A), vt_ai = v_ai/s_a,
  d_ai = u_i - c_a vt_ai, w_a = sum_i d^2, out(b,a) = sum_pix sqrt(w) mask.
Host multiplies per-(b,a) sums by |s_a|, applies gating + normalization.

Key identity: c_a*vt_ai = alpha[a,i]*(c_a Q_A) + beta[a,i]*(c_a Q_B)
              + h[a,i]*c_a, and h*c folds into the P0 weights (c = G p + g0).
So the only per-pixel elementwise product is y = crep2 (.) [Q_A; Q_B]
(PSUM x SBUF, one DVE op); everything else is matmuls.

Per chunk of 512 pixels (32 chunks/core), channel-major rows cls*8+b:
  MM_c: [25->48]  crep2 rows (side, a, b)       from P0+ones (XP tile)
  DVE:  y = crep2 (.) XQ[0:48] -> RHSD[0:48]    (Q_A | Q_B channel blocks)
  MM_d: [89->72]  d = u' - alpha y_A - beta y_B  (y + pad + P0 + ones)
  ACT:  SQ = Square(d) -> bf16
  MM_r: [72->128] w_a rows 32*slot + a*8 + b; 4 chunks accumulate into one
        [128,512] PSUM bank via slot-shifted weight matrices (start/stop)
  per group: ACT Sqrt -> la; DVE tensor_tensor_reduce(la*mask) -> ACC[:, g]

All matmuls are plain tile_position=(0,0); engine APs all base-0
(non-zero partition bases / col-group tiling are broken on this setup).
"""
import os
import sys

import numpy as np

for _p in ("/opt/trn_rl_repo",):
    if _p not in sys.path:
        sys.path.insert(0, _p)

import concourse.bass as bass
import concourse.bacc as bacc
import concourse.tile as tile
from concourse import mybir
from concourse.bass_utils import run_bass_kernel_spmd

from ml_dtypes import bfloat16

F32 = mybir.dt.float32
BF16 = mybir.dt.bfloat16
AF = mybir.ActivationFunctionType
OP = mybir.AluOpType

B, HW = 64, 128 * 128
NCORES, BPC = 8, 8
NCH = 512                  # pixels per chunk (one PSUM bank of fp32)
NCHUNKS = HW // NCH        # 32 chunks per core
NG = NCHUNKS // 4          # 8 groups of 4 chunks

# a -> (c1, c2, qA, qB): Q channel indices per direction
QCH = {0: (1, 2, 0, 1), 1: (0, 2, 2, 3), 2: (0, 1, 4, 5)}
QA_ORDER = [QCH[a][2] for a in range(3)]   # [0, 2, 4]
QB_ORDER = [QCH[a][3] for a in range(3)]   # [1, 3, 5]

_BUILT = None
LAST = None


def _build_nc():
    nc = bacc.Bacc(None)
    xq = nc.dram_tensor("xq", [48, HW], BF16, kind="ExternalInput")
    xz = nc.dram_tensor("xz", [25, HW], BF16, kind="ExternalInput")
    mk = nc.dram_tensor("mk", [128, NG * NCH], BF16, kind="ExternalInput")
    wc_d = nc.dram_tensor("wc", [25, 48], BF16, kind="ExternalInput")
    wd_d = nc.dram_tensor("wd", [73, 72], BF16, kind="ExternalInput")
    wr_d = nc.dram_tensor("wr", [72, 512], BF16, kind="ExternalInput")
    outp = nc.dram_tensor("out", [128, NG], F32, kind="ExternalOutput")

    with tile.TileContext(nc) as tc:
        with tc.tile_pool(name="big", bufs=1) as big, \
             tc.tile_pool(name="sq", bufs=4) as sqp, \
             tc.tile_pool(name="la", bufs=2) as lap, \
             tc.tile_pool(name="lm", bufs=2) as lmp, \
             tc.tile_pool(name="pc", bufs=3, space="PSUM") as pcp, \
             tc.tile_pool(name="pd", bufs=3, space="PSUM") as pdp, \
             tc.tile_pool(name="pw", bufs=2, space="PSUM") as pwp:
            WT = big.tile([73, 72 + 48], BF16, tag="WT")
            nc.scalar.dma_start(WT[0:73, 0:72], wd_d[:])
            nc.scalar.dma_start(WT[0:25, 72:120], wc_d[:])
            WRT = big.tile([72, 512], BF16, tag="WRT")
            nc.scalar.dma_start(WRT[:], wr_d[:])

            XQ = big.tile([48, HW], BF16, tag="XQ")
            XP = big.tile([25, HW], BF16, tag="XP")
            RHSD = big.tile([73, HW], BF16, tag="RHSD")
            MASK = big.tile([128, NG * NCH], BF16, tag="MASK")
            # column-piece loads so chunk-0 compute starts early and DMA
            # overlaps compute; alternate the two HWDGE rings
            NP = 16
            PW = HW // NP
            for p in range(NP):
                ps = slice(p * PW, (p + 1) * PW)
                e0 = nc.sync if p % 2 == 0 else nc.scalar
                e1 = nc.scalar if p % 2 == 0 else nc.sync
                e0.dma_start(XP[:, ps], xz[:, ps])
                e1.dma_start(RHSD[48:73, ps], xz[:, ps])
                e0.dma_start(XQ[:, ps], xq[:, ps])
                if p % 2 == 0:
                    ms = slice((p // 2) * NCH, (p // 2 + 1) * NCH)
                    e1.dma_start(MASK[:, ms], mk[:, ms])
            ACC = big.tile([128, NG], F32, tag="ACC")
            TTD = big.tile([128, NCH], BF16, tag="TTD")

            Wd = WT[0:73, 0:72]
            Wc = WT[0:25, 72:120]

            # software-pipelined emission (2-chunk skew) so each PE
            # instruction's inputs are ready well ahead -> PE runs dense,
            # HAM un-throttles to 2.4 GHz
            w = None
            sqt = [None] * NCHUNKS
            dt_ = [None] * NCHUNKS
            for t in range(NCHUNKS + 2):
                if t < NCHUNKS:
                    c = t
                    cs = slice(c * NCH, (c + 1) * NCH)
                    C = pcp.tile([48, NCH], F32, tag="C")
                    nc.tensor.matmul(C[:], Wc, XP[:, cs], start=True,
                                     stop=True)
                    nc.vector.tensor_mul(RHSD[0:48, cs], C[:], XQ[:, cs])
                if 1 <= t <= NCHUNKS:
                    c = t - 1
                    cs = slice(c * NCH, (c + 1) * NCH)
                    D = pdp.tile([72, NCH], F32, tag="D")
                    dt_[c] = D
                    nc.tensor.matmul(D[:], Wd, RHSD[:, cs], start=True,
                                     stop=True)
                    sq = sqp.tile([72, NCH], BF16, tag="sq")
                    sqt[c] = sq
                    nc.scalar.activation(sq[:], D[:], AF.Square)
                if t >= 2:
                    c = t - 2
                    g, slot = divmod(c, 4)
                    gs = slice(g * NCH, (g + 1) * NCH)
                    if slot == 0:
                        w = pwp.tile([128, NCH], F32, tag="W")
                    nc.tensor.matmul(w[:], WRT[:, 128 * slot:128 * slot + 128],
                                     sqt[c][:], start=(slot == 0),
                                     stop=(slot == 3))
                    sqt[c] = None
                    if slot == 3:
                        la = lap.tile([128, NCH], BF16, tag="la")
                        nc.scalar.activation(la[:], w[:], AF.Sqrt)
                        lm = lmp.tile([128, NCH], BF16, tag="lm")
                        nc.vector.tensor_mul(lm[:], la[:], MASK[:, gs])
                        nc.scalar.activation(TTD[:], lm[:], AF.Identity,
                                             accum_out=ACC[:, g:g + 1])

            nc.sync.dma_start(outp[:], ACC[:])

    nc.compile()
    return nc


def get_nc():
    global _BUILT
    if _BUILT is None:
        _BUILT = _build_nc()
    return _BUILT


def host_constants(R, T, E):
    """Per-core weight matrices (fp64 host math -> bf16)."""
    wc = np.zeros((NCORES, 25, 48), np.float64)
    wd = np.zeros((NCORES, 73, 72), np.float64)
    wr = np.zeros((NCORES, 72, 512), np.float64)
    sabs = np.zeros((B, 3), np.float64)
    for gb in range(B):
        k, b = divmod(gb, BPC)
        Rb = R[gb].astype(np.float64)
        tb = T[gb].astype(np.float64)
        eb = E[gb].astype(np.float64)
        A = Rb * eb[None, :]
        b0 = tb - 0.5 * (Rb @ eb)
        G = Rb.T @ A
        g0 = Rb.T @ b0
        s = Rb.T @ tb
        for a in range(3):
            c1, c2, _, _ = QCH[a]
            sh = np.sign(s[a]) * max(abs(s[a]), 1e-12) if s[a] != 0 else 1e-12
            sabs[gb, a] = abs(sh)
            # crep2 columns: side*24 + a*8 + b
            for side in range(2):
                r2 = side * 24 + a * 8 + b
                for j in range(3):
                    wc[k, j * 8 + b, r2] = G[a, j]
                wc[k, 24, r2] = g0[a]
            for i in range(3):
                r = (3 * a + i) * 8 + b
                al = A[i, c1] / sh
                be = A[i, c2] / sh
                h = (tb[i] - 0.5 * (A[i, c1] + A[i, c2])) / sh
                wd[k, a * 8 + b, r] = -al
                wd[k, 24 + a * 8 + b, r] = -be
                for j in range(3):
                    wd[k, 48 + j * 8 + b, r] = A[i, j] - h * G[a, j]
                wd[k, 72, r] = b0[i] - h * g0[a]
                for slot in range(4):
                    wr[k, r, 128 * slot + 32 * slot + a * 8 + b] = 1.0
    return (wc.astype(bfloat16), wd.astype(bfloat16),
            wr.astype(bfloat16), sabs)


def make_in_maps(P0, Q0, M, wc, wd, wr):
    in_maps = []
    for k in range(NCORES):
        sl = slice(k * BPC, (k + 1) * BPC)
        # xq rows: 0..23 Q_A channels [0,2,4] (block a), 24..47 Q_B [1,3,5]
        q = Q0[sl].reshape(BPC, 6, HW)
        xq = np.empty((48, HW), np.float32)
        xq[0:24] = q[:, QA_ORDER].transpose(1, 0, 2).reshape(24, HW)
        xq[24:48] = q[:, QB_ORDER].transpose(1, 0, 2).reshape(24, HW)
        # xz: rows 0..23 P0 rows j*8+b, 24 ones
        xz = np.zeros((25, HW), np.float32)
        xz[0:24] = P0[sl].reshape(BPC, 3, HW).transpose(1, 0, 2).reshape(24, HW)
        xz[24] = 1.0
        # mask rows 32*slot + a*8 + b, cols g*NCH+p <- M[b,a,(4g+slot)*NCH+p]
        mkc = np.zeros((4, 32, NG, NCH), np.float32)  # slot, row, g, p
        msl = M[sl].reshape(BPC, 3, NG, 4, NCH)       # b a g slot p
        mkc[:, 0:24] = msl.transpose(3, 1, 0, 2, 4).reshape(4, 24, NG, NCH)
        mk = mkc.reshape(128, NG * NCH)
        in_maps.append({
            "xq": xq.astype(bfloat16), "xz": xz.astype(bfloat16),
            "mk": mk.astype(bfloat16),
            "wc": np.ascontiguousarray(wc[k]),
            "wd": np.ascontiguousarray(wd[k]),
            "wr": np.ascontiguousarray(wr[k]),
        })
    return in_maps


def kernel(pred_rots, pred_P0, pred_Q0, gt_occmask, roi_extent, pred_transes):
    global LAST
    R = np.asarray(pred_rots, np.float32)
    P0 = np.asarray(pred_P0, np.float32)
    Q0 = np.asarray(pred_Q0, np.float32)
    M = np.asarray(gt_occmask, np.float32)
    E = np.asarray(roi_extent, np.float32)
    T = np.asarray(pred_transes, np.float32)

    nc = get_nc()
    wc, wd, wr, sabs = host_constants(R, T, E)
    in_maps = make_in_maps(P0, Q0, M, wc, wd, wr)
    trace = os.environ.get("KERNEL_TRACE", "0") == "1"
    LAST = run_bass_kernel_spmd(nc, in_maps, core_ids=list(range(NCORES)),
                                trace=trace)
    S = np.zeros((B, 3), np.float64)
    for k, r in enumerate(LAST.results):
        acc = r["out"].astype(np.float64).sum(axis=1).reshape(4, 32)
        rows = acc[:, 0:24].sum(axis=0)              # over slots
        S[k * BPC:(k + 1) * BPC] += rows.reshape(3, 8).T   # b, a
    S *= sabs
    Msum_a = M.sum(axis=(0, 2, 3)).astype(np.float64)  # per-a mask sums
    loss = sum(S[:, a].sum() for a in range(3) if Msum_a[a] >= 3 * B)
    total = max(Msum_a.sum(), 1.0)
    return np.asarray(np.float32(loss / total))


# revision 38
# speedup vs baseline: 1.4570x; 1.3535x over previous
"""Trainium2 Bass kernel for nn_CT_loss (data-parallel over batch, 8 cores).

Math (R is a general 3x3 matrix, not orthogonal):
  u   = A P0 + b0          A = R diag(e), b0 = t - 0.5 R e      (per batch)
  c   = G P0 + g0          G = R^T A,     g0 = R^T b0
  v_a = A[:,c1] Qa' + A[:,c2] Qb' + h_a  (Q' = Q-0.5), s = R^T t
  d_a = s_a u - c_a v_a ;  la = sqrt(|d_a|^2 m_a)
  loss = sum_a [sum(m_a) >= 3B] sum(la) / max(sum_a sum(m_a), 1)

Device trick 1: fold 1/s_a into v's affine coefficients (vt = v/s_a), so
  d~_a = u - c_a vt_a  is scalar-free; host multiplies the per-batch
  partial sums by |s_a| during the gather.
Device trick 2: avoid scalar_tensor_tensor entirely (no fast DVE uop, 1x):
  every op is tensor_scalar/activation (1-src affine, 2x/1x) or
  tensor_tensor (2x bf16) combining them.

Layout per core: 8 batches; tiles [128, FD=1024]; partition = b*16+g,
free = 1024 pixels. Per-batch scalars ride as per-partition [128,1]
columns of a constants tile. Free-dim sums via accum_out; host finishes
the 128-row + cross-core reduction (the "gather").
"""
import os
import sys

import numpy as np

for _p in ("/opt/trn_rl_repo",):
    if _p not in sys.path:
        sys.path.insert(0, _p)

import concourse.bass as bass
import concourse.bacc as bacc
import concourse.tile as tile
from concourse import mybir
from concourse.bass_utils import run_bass_kernel_spmd

from ml_dtypes import bfloat16

F32 = mybir.dt.float32
BF16 = mybir.dt.bfloat16
AF = mybir.ActivationFunctionType
OP = mybir.AluOpType

B, HW = 64, 128 * 128
NCORES, BPC, G, FD = 8, 8, 16, 1024
F3 = 3 * FD

# a -> (Acol1, Acol2, qchA, qchB)
QCH = {0: (1, 2, 0, 1), 1: (0, 2, 2, 3), 2: (0, 1, 4, 5)}

# constants tile columns
CA = 0    # A[i*3+j] 9
CB0 = 9   # b0 3
CG = 12   # G[a*3+j] 9
CG0 = 21  # g0 3
CV1 = 24  # alpha~[a*3+i] = A[i,c1]/s~_a 9
CHC = 33  # h~[a*3+i] 9
CV2 = 42  # beta~[a*3+i] = A[i,c2]/s~_a 9
CZ = 51   # 0.0 (zero bias so ACT terms can always use Identity)
NCST = 52

# engine for each 1-src scaled-term family: u terms, c terms, v1, v2
E_TERMS = {"u": "act", "c": "act", "v1": "act", "v2": "vec"}
E_SQ_A = ["act", "act", "vec"]  # squares engine per a
E_MSUM = "act"    # mask sums via activation accum
E_W = "vec"       # w = sq0+sq1+sq2
E_WM = "vec"      # w *= mask

_BUILT = None
LAST = None


def _term(nc, eng, out, in_, sc, bi):
    """out = in_*sc + bi, per-partition scalar APs (bi required)."""
    if eng == "act":
        nc.scalar.activation(out, in_, AF.Identity, bias=bi, scale=sc)
    else:
        e = nc.vector if eng == "vec" else nc.gpsimd
        e.tensor_scalar(out, in_, sc, bi, op0=OP.mult, op1=OP.add)


def _eng(nc, eng):
    return nc.vector if eng == "vec" else nc.gpsimd


def _bcast3(ap, n):
    """[128, FD] AP -> [128, n, FD] with step-0 middle dim."""
    return bass.AP(tensor=ap.tensor, offset=ap.offset,
                   ap=[ap.ap[0], [0, n], *ap.ap[1:]])


def _build_nc():
    nc = bacc.Bacc(None)
    p0 = nc.dram_tensor("p0", [BPC, G, 3, FD], BF16, kind="ExternalInput")
    q0 = nc.dram_tensor("q0", [BPC, G, 6, FD], BF16, kind="ExternalInput")
    mk = nc.dram_tensor("mk", [BPC, G, 3, FD], BF16, kind="ExternalInput")
    cst = nc.dram_tensor("cst", [128, NCST], F32, kind="ExternalInput")
    outp = nc.dram_tensor("out", [128, 3], F32, kind="ExternalOutput")

    with tile.TileContext(nc) as tc:
        with tc.tile_pool(name="main", bufs=1) as pool, \
             tc.tile_pool(name="terms", bufs=6) as terms:
            # two HWDGE rings: sync gets p0+mk, scalar gets cst+q0
            cst_t = pool.tile([128, NCST], F32, tag="cst")
            nc.scalar.dma_start(cst_t[:], cst[:])

            def cs(j):
                return cst_t[:, j:j + 1]

            warm = pool.tile([128, 1], BF16, tag="warm")
            nc.scalar.activation(warm[:], cst_t[:, CZ:CZ + 1], AF.Sqrt)

            p0_t = pool.tile([128, 3, FD], BF16, tag="p0")
            p0r = p0[:].rearrange("b g c f -> (b g) c f")
            nc.sync.dma_start(p0_t[:, 0:2, :], p0r[:, 0:2, :])
            nc.scalar.dma_start(p0_t[:, 2:3, :], p0r[:, 2:3, :])
            q0_t = pool.tile([128, 6, FD], BF16, tag="q0")
            q0r = q0[:].rearrange("b g c f -> (b g) c f")
            for cc in range(3):
                nc.scalar.dma_start(q0_t[:, 2 * cc:2 * cc + 2, :],
                                    q0r[:, 2 * cc:2 * cc + 2, :])
            mk_t = pool.tile([128, 3, FD], BF16, tag="mk")
            nc.sync.dma_start(mk_t[:], mk[:].rearrange("b g c f -> (b g) c f"))

            acc = pool.tile([128, 3], F32, tag="acc")

            X = [p0_t[:, j, :] for j in range(3)]
            Q = [q0_t[:, j, :] for j in range(6)]
            MSK = [mk_t[:, a, :] for a in range(3)]

            zero = cs(CZ)

            def lin3(eng, outs, srcs, csc, cbi):
                for k in range(3):
                    t2 = terms.tile([128, FD], BF16, name="t2x", tag="t2")
                    _term(nc, eng[0], t2, srcs[2], csc(k, 2), cbi(k))
                    t1 = terms.tile([128, FD], BF16, name="t1x", tag="t1")
                    _term(nc, eng[1], t1, srcs[1], csc(k, 1), zero)
                    t0 = terms.tile([128, FD], BF16, name="t0x", tag="t0")
                    _term(nc, eng[2], t0, srcs[0], csc(k, 0), zero)
                    nc.vector.tensor_add(outs[k], t2, t1)
                    nc.vector.tensor_add(outs[k], outs[k], t0)

            u3 = pool.tile([128, 3, FD], BF16, tag="u3")
            lin3(["act", "vec", "act"], [u3[:, i, :] for i in range(3)],
                 X, lambda i, j: cs(CA + 3 * i + j), lambda i: cs(CB0 + i))
            c3 = pool.tile([128, 3, FD], BF16, tag="c3")
            lin3(["vec", "act", "vec"], [c3[:, a, :] for a in range(3)],
                 X, lambda a, j: cs(CG + 3 * a + j), lambda a: cs(CG0 + a))
            c_t = [c3[:, a, :] for a in range(3)]

            # mask sums are computed host-side (numpy) from the raw input
            vas = []
            for a in range(3):
                c1, c2, qA, qB = QCH[a]
                va = pool.tile([128, 3, FD], BF16, name=f"va{a}", tag=f"va{a}")
                vas.append(va)
                for i in range(3):
                    tv1 = terms.tile([128, FD], BF16, name="tv1x", tag="tv1")
                    _term(nc, E_TERMS["v1"], tv1, Q[qA], cs(CV1 + 3 * a + i),
                          cs(CHC + 3 * a + i))
                    tv2 = terms.tile([128, FD], BF16, name="tv2x", tag="tv2")
                    _term(nc, E_TERMS["v2"], tv2, Q[qB], cs(CV2 + 3 * a + i),
                          zero)
                    nc.vector.tensor_add(va[:, i, :], tv1, tv2)
                nc.vector.tensor_mul(va[:], _bcast3(c_t[a], 3), va[:])
                nc.vector.tensor_sub(va[:], u3[:], va[:])
                sq = pool.tile([128, 3, FD], BF16, name=f"sq{a}", tag=f"sq{a}")
                if E_SQ_A[a] == "act":
                    nc.scalar.activation(sq[:], va[:], AF.Square)
                else:
                    nc.vector.tensor_mul(sq[:], va[:], va[:])
                vas[a] = sq
            for a in range(3):
                sq = vas[a]
                w = pool.tile([128, FD], BF16, name=f"w{a}", tag=f"w{a}")
                nc.vector.tensor_add(w, sq[:, 0, :], sq[:, 1, :])
                nc.vector.tensor_add(w, w, sq[:, 2, :])
                nc.vector.tensor_mul(w, w, MSK[a])
                la = pool.tile([128, FD], BF16, name=f"la{a}", tag="la")
                nc.scalar.activation(la, w, AF.Sqrt, accum_out=acc[:, a:a + 1])

            nc.sync.dma_start(outp[:], acc[:])

    nc.compile()
    return nc


def get_nc():
    global _BUILT
    if _BUILT is None:
        _BUILT = _build_nc()
    return _BUILT


def host_constants(R, T, E):
    """[B, NCST] fp32 constants (fp64 host math) + [B,3] |s| scales."""
    Bn = R.shape[0]
    out = np.zeros((Bn, NCST), np.float64)
    sabs = np.zeros((Bn, 3), np.float64)
    for b in range(Bn):
        Rb = R[b].astype(np.float64)
        tb = T[b].astype(np.float64)
        eb = E[b].astype(np.float64)
        A = Rb * eb[None, :]
        b0 = tb - 0.5 * (Rb @ eb)
        Gm = Rb.T @ A
        g0 = Rb.T @ b0
        s = Rb.T @ tb
        out[b, CA:CA + 9] = A.reshape(-1)
        out[b, CB0:CB0 + 3] = b0
        out[b, CG:CG + 9] = Gm.reshape(-1)
        out[b, CG0:CG0 + 3] = g0
        for a, (c1, c2, _, _) in QCH.items():
            sh = np.sign(s[a]) * max(abs(s[a]), 1e-12) if s[a] != 0 else 1e-12
            sabs[b, a] = abs(s[a])
            h = tb - 0.5 * (A[:, c1] + A[:, c2])
            out[b, CV1 + 3 * a:CV1 + 3 * a + 3] = A[:, c1] / sh
            out[b, CV2 + 3 * a:CV2 + 3 * a + 3] = A[:, c2] / sh
            out[b, CHC + 3 * a:CHC + 3 * a + 3] = h / sh
    return out.astype(np.float32), sabs


def make_in_maps(P0, Q0, M, cst):
    in_maps = []
    for k in range(NCORES):
        sl = slice(k * BPC, (k + 1) * BPC)
        in_maps.append({
            "p0": P0[sl].reshape(BPC, 3, G, FD).transpose(0, 2, 1, 3).astype(bfloat16),
            "q0": Q0[sl].reshape(BPC, 6, G, FD).transpose(0, 2, 1, 3).astype(bfloat16),
            "mk": M[sl].reshape(BPC, 3, G, FD).transpose(0, 2, 1, 3).astype(bfloat16),
            "cst": np.ascontiguousarray(np.repeat(cst[sl], G, axis=0)),
        })
    return in_maps


def kernel(pred_rots, pred_P0, pred_Q0, gt_occmask, roi_extent, pred_transes):
    global LAST
    R = np.asarray(pred_rots, np.float32)
    P0 = np.asarray(pred_P0, np.float32)
    Q0 = np.asarray(pred_Q0, np.float32)
    M = np.asarray(gt_occmask, np.float32)
    E = np.asarray(roi_extent, np.float32)
    T = np.asarray(pred_transes, np.float32)

    nc = get_nc()
    cst, sabs = host_constants(R, T, E)
    in_maps = make_in_maps(P0, Q0, M, cst)
    trace = os.environ.get("KERNEL_TRACE", "0") == "1"
    LAST = run_bass_kernel_spmd(nc, in_maps, core_ids=list(range(NCORES)),
                                trace=trace)
    S_a = np.zeros(3, np.float64)
    for k, r in enumerate(LAST.results):
        o = r["out"].astype(np.float64)          # [128, 3]
        st = o.reshape(BPC, G, 3).sum(axis=1)    # [BPC, 3] per-batch
        S_a += (st * sabs[k * BPC:(k + 1) * BPC]).sum(axis=0)
    M_a = M.sum(axis=(0, 2, 3)).astype(np.float64)
    loss = sum(0.0 if M_a[a] < 3 * B else S_a[a] for a in range(3))
    total = max(M_a.sum(), 1.0)
    return np.asarray(np.float32(loss / total))

